# revision 6
# baseline (speedup 1.0000x reference)
"""Trainium2 Bass kernel for BaseGenerator: mapped = mapping @ base_flat.

Strategy (8-core SPMD, pure data-parallel over output pixels):
  - mapping [P1=16384, P0=16384] f32 is row-sharded: core c owns output rows
    [c*2048, (c+1)*2048).  Host pre-transposes each shard to mt_c [P0, 2048]
    (K-major, cast to fp8 e4m3) so the contraction axis lands on SBUF
    partitions and the device streams the shard with contiguous multi-MiB
    DMAs plus a fine-grained tail (so the final matmuls chase the stream).
  - base_flat [P0, 3] is replicated, rearranged host-side to
    [128, 128 chunks, 16] (3 channels + 13 pad) so each 128-row K-chunk
    gives a [128, 3] stationary matmul operand with a 16 B chunk stride
    (DoubleRow requires the pair-dim step to be a multiple of 16 B).
  - Device (fp8 path): pairs of K-chunks feed DoubleRow matmuls
    (lhsT [128,2,3], rhs [128,2,512] -> out [3,512], K_eff=256/instr, 2 fp8
    weights per PE cell) accumulating into 4 persistent PSUM banks of
    [3, 512] f32 across all 64 chunk pairs.  Epilogue copies
    PSUM -> SBUF -> DRAM out [3, 2048] f32.
  - Host concatenates per-core outputs -> [16384, 3] -> [128, 128, 3].

The kernel is DMA-bound: 32 MiB/core (8-bit) streams at ~400 GB/s/core,
which saturates chip HBM across 8 cores.  COMPUTE_DTYPE selects the
internal precision of the mapping/base operands: "float8e4" (shipped)
-> absmax rel err ~1.3e-3 vs the f32 reference at ~2x the fp16 speed;
"float16" -> 1.1e-5 at ~181 us.  Accumulation is always f32 in PSUM;
output is always f32.
"""

import sys

import numpy as np

try:
    import concourse.bacc as bacc
except ImportError:  # fresh env without PYTHONPATH: fall back to repo paths
    for _p in ("/opt/trn_rl_repo", "/opt/pypackages",
               "/root/.axon_site/_ro/trn_rl_repo",
               "/root/.axon_site/_ro/pypackages"):
        if _p not in sys.path:
            sys.path.append(_p)
    import concourse.bacc as bacc
import concourse.bass as bass
import concourse.mybir as mybir
import concourse.tile as tile
from concourse.bass_utils import run_bass_kernel_spmd

H0 = W0 = 128
H1 = W1 = 128
P0 = H0 * W0          # 16384 contraction length
P1 = H1 * W1          # 16384 output pixels
N_CORES = 8
N_PER_CORE = P1 // N_CORES   # 2048 output pixels per core
KC = 128              # K-chunk size (SBUF partitions)
N_KCHUNKS = P0 // KC  # 128
NB = 512              # matmul moving free dim (one PSUM bank of f32)
N_BANKS = N_PER_CORE // NB   # 4
BPAD = 16             # fp8 base chunk slot (3 used + 13 pad; 16 B pair stride)

COMPUTE_DTYPE = "float8e4"   # e4m3: halves DMA bytes vs fp16; rel err ~1.3e-3
CHUNKS_PER_DMA = 4           # K-chunks fetched per dma_start (4 -> 1 MiB fp8)
DMA_BUFS = 8                 # in-flight DMA tiles
RAMP = 2                     # chunks per DMA piece during startup ramp
TAILQ = 2                    # chunks per DMA piece in the tail

_PROGRAM_CACHE = {}


def _np_compute_dtype(name):
    if name == "float32":
        return np.float32
    if name == "float16":
        return np.float16
    import ml_dtypes
    if name == "float8e4":
        return ml_dtypes.float8_e4m3
    return ml_dtypes.bfloat16


def _build_program(dtype_name):
    """Build + compile the SPMD Bass program (identical on all 8 cores)."""
    dt = getattr(mybir.dt, dtype_name)
    fp8 = dtype_name == "float8e4"
    bw = BPAD if fp8 else 3
    nc = bacc.Bacc(
        "TRN2", target_bir_lowering=False, debug=False, num_devices=N_CORES
    )
    qc = CHUNKS_PER_DMA
    n_dmas = N_KCHUNKS // qc
    mt = nc.dram_tensor("mt", [n_dmas * KC, qc, N_PER_CORE], dt,
                        kind="ExternalInput")
    bt = nc.dram_tensor("bt", [KC, N_KCHUNKS, bw], dt, kind="ExternalInput")
    out = nc.dram_tensor(
        "out", [3, N_PER_CORE], mybir.dt.float32, kind="ExternalOutput"
    )

    # mt[(i*KC)+p, a, n]: tile i, partition p, chunk-in-tile a, out col n --
    # each DMA tile is one contiguous qc*N_PER_CORE read per partition.
    with tile.TileContext(nc) as tc:
        with (
            tc.tile_pool(name="bpool", bufs=1) as bpool,
            tc.tile_pool(name="mpool", bufs=DMA_BUFS) as mpool,
            tc.tile_pool(name="psum", bufs=1, space=bass.MemorySpace.PSUM) as pp,
            tc.tile_pool(name="opool", bufs=1) as opool,
        ):
            b_sb = bpool.tile([KC, N_KCHUNKS, bw], dt)
            nc.sync.dma_start(b_sb[:], bt[:])

            ps = [
                pp.tile([3, NB], mybir.dt.float32, name=f"ps{i}", tag=f"ps{i}")
                for i in range(N_BANKS)
            ]

            if fp8:
                n_pairs = N_KCHUNKS // 2

                def tile_mms(m_tile, kp0, a0, a_cnt):
                    # DoubleRow: each matmul consumes chunk pair (2kp, 2kp+1);
                    # lhsT [128,2,3], rhs [128,2,NB] -> out [3,NB].
                    for a2 in range(a_cnt // 2):
                        kp = kp0 + a2
                        lhsT = b_sb[:, 2 * kp:2 * kp + 2, 0:3]
                        for nb in range(N_BANKS):
                            nc.tensor.matmul(
                                ps[nb][:, :],
                                lhsT,
                                m_tile[:, (a0 + 2 * a2):(a0 + 2 * a2 + 2),
                                       nb * NB:(nb + 1) * NB],
                                start=(kp == 0),
                                stop=(kp == n_pairs - 1),
                                perf_mode=mybir.MatmulPerfMode.DoubleRow,
                            )
            else:

                def tile_mms(m_tile, k0, a0, a_cnt):
                    for a in range(a_cnt):
                        lhsT = b_sb[:, k0 + a, 0:3]
                        for nb in range(N_BANKS):
                            nc.tensor.matmul(
                                ps[nb][:, :],
                                lhsT,
                                m_tile[:, a0 + a, nb * NB:(nb + 1) * NB],
                                start=(k0 + a == 0),
                                stop=(k0 + a == N_KCHUNKS - 1),
                            )

            # DMA schedule over the qc-chunk dram tiles: a short ramp of
            # small (sub-tile) DMAs so the first matmuls start as early as
            # possible, full qc-chunk tiles for the bulk, and a fine-grained
            # tail so the last matmuls chase the stream.
            # Each entry: (dram_tile_idx, chunk_lo, n_chunks, pool_tag).
            sched = []
            for j in range(qc // RAMP):           # ramp: tile 0 in pieces
                sched.append((0, j * RAMP, RAMP, "m_rp"))
            for i in range(1, n_dmas - 1):        # bulk
                sched.append((i, 0, qc, "m_sb"))
            for j in range(qc // TAILQ):          # tail: last tile in pieces
                sched.append((n_dmas - 1, j * TAILQ, TAILQ, "m_tl"))

            pool_bufs = {"m_rp": qc // RAMP, "m_sb": DMA_BUFS,
                         "m_tl": qc // TAILQ}
            for ti, c_lo, c_n, tag in sched:
                m_sb = mpool.tile([KC, c_n, N_PER_CORE], dt, name=tag,
                                  tag=tag, bufs=pool_bufs[tag])
                nc.sync.dma_start(
                    m_sb[:], mt[ti * KC:(ti + 1) * KC, c_lo:c_lo + c_n]
                )
                k0 = ti * qc + c_lo
                if fp8:
                    tile_mms(m_sb, k0 // 2, 0, c_n)
                else:
                    tile_mms(m_sb, k0, 0, c_n)

            # Epilogue: PSUM -> SBUF on two engines in parallel, then one DMA.
            o_sb = opool.tile([3, N_PER_CORE], mybir.dt.float32)
            for nb in range(N_BANKS):
                dst = o_sb[:, nb * NB:(nb + 1) * NB]
                if nb % 2 == 0:
                    nc.vector.tensor_copy(dst, ps[nb][:, :])
                else:
                    nc.scalar.copy(dst, ps[nb][:, :])
            nc.sync.dma_start(out[:], o_sb[:])

    nc.compile()
    return nc


def _get_program(dtype_name):
    if dtype_name not in _PROGRAM_CACHE:
        _PROGRAM_CACHE[dtype_name] = _build_program(dtype_name)
    return _PROGRAM_CACHE[dtype_name]


def _prepare_inputs(mapping, base_image, dtype_name):
    np_dt = _np_compute_dtype(dtype_name)
    fp8 = dtype_name == "float8e4"
    bw = BPAD if fp8 else 3
    # base [128,128,3] -> base_flat [P0, 3] -> [128 part, 128 kchunk, bw]
    # bt[p, k1, c] = base_flat[k1*128 + p, c]  (c < 3; rest pad)
    base_flat = np.asarray(base_image, dtype=np.float32).reshape(P0, 3)
    bt = np.zeros((KC, N_KCHUNKS, bw), dtype=np_dt)
    bt[:, :, :3] = base_flat.reshape(N_KCHUNKS, KC, 3).transpose(1, 0, 2)

    qc = CHUNKS_PER_DMA
    n_t = N_KCHUNKS // qc
    in_maps = []
    for c in range(N_CORES):
        shard = mapping[c * N_PER_CORE:(c + 1) * N_PER_CORE, :]  # [2048, P0]
        mt_c = shard.T.astype(np_dt)  # [P0, 2048] K-major
        # tile-major: [tile i][partition p][chunk a][n] so each DMA tile is
        # one contiguous qc*2048 B read per partition
        mt_c = np.ascontiguousarray(
            mt_c.reshape(n_t, qc, KC, N_PER_CORE).swapaxes(1, 2)
        ).reshape(n_t * KC, qc, N_PER_CORE)
        in_maps.append({"mt": mt_c, "bt": bt})
    return in_maps


def _run(mapping, base_image, dtype_name, trace=False):
    nc = _get_program(dtype_name)
    in_maps = _prepare_inputs(mapping, base_image, dtype_name)
    res = run_bass_kernel_spmd(nc, in_maps, list(range(N_CORES)), trace=trace)
    mapped_flat = np.concatenate(
        [res.results[c]["out"].T for c in range(N_CORES)], axis=0
    )  # [P1, 3] f32
    mapped_image = mapped_flat.reshape(H1, W1, 3)
    return mapped_image, res


def kernel(mapping, base_image):
    mapping = np.asarray(mapping, dtype=np.float32)
    base_image = np.asarray(base_image, dtype=np.float32)
    mapped_image, _ = _run(mapping, base_image, COMPUTE_DTYPE)
    return (base_image, mapped_image)


# revision 11
# speedup vs baseline: 1.1517x; 1.1517x over previous
"""Trainium2 Bass kernel for BaseGenerator: mapped = mapping @ base_flat.

Strategy (8-core SPMD, pure data-parallel over output pixels):
  - mapping [P1=16384, P0=16384] f32 is row-sharded: core c owns output rows
    [c*2048, (c+1)*2048).  Host pre-transposes each shard to mt_c [P0, 2048]
    (K-major, cast to fp8 e4m3) so the contraction axis lands on SBUF
    partitions and the device streams the shard with contiguous multi-MiB
    DMAs plus a fine-grained tail (so the final matmuls chase the stream).
  - base_flat [P0, 3] is replicated, rearranged host-side to
    [128, 128 chunks, 16] (3 channels + 13 pad) so each 128-row K-chunk
    gives a [128, 3] stationary matmul operand with a 16 B chunk stride
    (DoubleRow requires the pair-dim step to be a multiple of 16 B).
  - Device (fp8 path): pairs of K-chunks feed DoubleRow matmuls
    (lhsT [128,2,3], rhs [128,2,512] -> out [3,512], K_eff=256/instr, 2 fp8
    weights per PE cell) accumulating into 4 persistent PSUM banks of
    [3, 512] f32 across all 64 chunk pairs.  Epilogue copies
    PSUM -> SBUF -> DRAM out [3, 2048] f32.
  - Host concatenates per-core outputs -> [16384, 3] -> [128, 128, 3].

The kernel is DMA-bound: 32 MiB/core (8-bit) streams at ~400 GB/s/core,
which saturates chip HBM across 8 cores.  COMPUTE_DTYPE selects the
internal precision of the mapping/base operands: "float8e4" (shipped)
-> absmax rel err ~1.3e-3 vs the f32 reference at ~2x the fp16 speed;
"float16" -> 1.1e-5 at ~181 us.  Accumulation is always f32 in PSUM;
output is always f32.
"""

import sys

import numpy as np

try:
    import concourse.bacc as bacc
except ImportError:  # fresh env without PYTHONPATH: fall back to repo paths
    for _p in ("/opt/trn_rl_repo", "/opt/pypackages",
               "/root/.axon_site/_ro/trn_rl_repo",
               "/root/.axon_site/_ro/pypackages"):
        if _p not in sys.path:
            sys.path.append(_p)
    import concourse.bacc as bacc
import concourse.bass as bass
import concourse.mybir as mybir
import concourse.tile as tile
from concourse.bass_utils import run_bass_kernel_spmd

H0 = W0 = 128
H1 = W1 = 128
P0 = H0 * W0          # 16384 contraction length
P1 = H1 * W1          # 16384 output pixels
N_CORES = 8
N_PER_CORE = P1 // N_CORES   # 2048 output pixels per core
KC = 128              # K-chunk size (SBUF partitions)
N_KCHUNKS = P0 // KC  # 128
NB = 512              # matmul moving free dim (one PSUM bank of f32)
N_BANKS = N_PER_CORE // NB   # 4
BPAD = 16             # fp8 base chunk slot (3 used + 13 pad; 16 B pair stride)

COMPUTE_DTYPE = "float8e4"   # e4m3: halves DMA bytes vs fp16; rel err ~1.3e-3
CHUNKS_PER_DMA = 4           # K-chunks fetched per dma_start (4 -> 1 MiB fp8)
DMA_BUFS = 12                # in-flight DMA tiles
RAMP = 2                     # chunks per DMA piece during startup ramp
TAILQ = 2                    # chunks per DMA piece in the tail

_PROGRAM_CACHE = {}


def _np_compute_dtype(name):
    if name == "float32":
        return np.float32
    if name == "float16":
        return np.float16
    import ml_dtypes
    if name == "float8e4":
        return ml_dtypes.float8_e4m3
    return ml_dtypes.bfloat16


def _build_program(dtype_name):
    """Build + compile the SPMD Bass program (identical on all 8 cores)."""
    dt = getattr(mybir.dt, dtype_name)
    fp8 = dtype_name == "float8e4"
    bw = BPAD if fp8 else 3
    nc = bacc.Bacc(
        "TRN2", target_bir_lowering=False, debug=False, num_devices=N_CORES
    )
    qc = CHUNKS_PER_DMA
    n_dmas = N_KCHUNKS // qc
    mt = nc.dram_tensor("mt", [n_dmas * KC, qc, N_PER_CORE], dt,
                        kind="ExternalInput")
    bt = nc.dram_tensor("bt", [KC, N_KCHUNKS, bw], dt, kind="ExternalInput")
    out = nc.dram_tensor(
        "out", [3, N_PER_CORE], mybir.dt.float32, kind="ExternalOutput"
    )

    # mt[(i*KC)+p, a, n]: tile i, partition p, chunk-in-tile a, out col n --
    # each DMA tile is one contiguous qc*N_PER_CORE read per partition.
    with tile.TileContext(nc) as tc:
        with (
            tc.tile_pool(name="bpool", bufs=1) as bpool,
            tc.tile_pool(name="mpool", bufs=DMA_BUFS) as mpool,
            tc.tile_pool(name="psum", bufs=1, space=bass.MemorySpace.PSUM) as pp,
            tc.tile_pool(name="opool", bufs=1) as opool,
        ):
            b_sb = bpool.tile([KC, N_KCHUNKS, bw], dt)

            ps = [
                pp.tile([3, NB], mybir.dt.float32, name=f"ps{i}", tag=f"ps{i}")
                for i in range(N_BANKS)
            ]

            if fp8:
                n_pairs = N_KCHUNKS // 2

                def tile_mms(m_tile, kp0, a0, a_cnt):
                    # DoubleRow: each matmul consumes chunk pair (2kp, 2kp+1);
                    # lhsT [128,2,3], rhs [128,2,NB] -> out [3,NB].
                    for a2 in range(a_cnt // 2):
                        kp = kp0 + a2
                        lhsT = b_sb[:, 2 * kp:2 * kp + 2, 0:3]
                        for nb in range(N_BANKS):
                            nc.tensor.matmul(
                                ps[nb][:, :],
                                lhsT,
                                m_tile[:, (a0 + 2 * a2):(a0 + 2 * a2 + 2),
                                       nb * NB:(nb + 1) * NB],
                                start=(kp == 0),
                                stop=(kp == n_pairs - 1),
                                perf_mode=mybir.MatmulPerfMode.DoubleRow,
                            )
            else:

                def tile_mms(m_tile, k0, a0, a_cnt):
                    for a in range(a_cnt):
                        lhsT = b_sb[:, k0 + a, 0:3]
                        for nb in range(N_BANKS):
                            nc.tensor.matmul(
                                ps[nb][:, :],
                                lhsT,
                                m_tile[:, a0 + a, nb * NB:(nb + 1) * NB],
                                start=(k0 + a == 0),
                                stop=(k0 + a == N_KCHUNKS - 1),
                            )

            # DMA schedule over the qc-chunk dram tiles: a short ramp of
            # small (sub-tile) DMAs so the first matmuls start as early as
            # possible, full qc-chunk tiles for the bulk, and a fine-grained
            # tail so the last matmuls chase the stream.
            # Each entry: (dram_tile_idx, chunk_lo, n_chunks, pool_tag).
            sched = []
            for j in range(qc // RAMP):           # ramp: tile 0 in pieces
                sched.append((0, j * RAMP, RAMP, "m_rp"))
            for i in range(1, n_dmas - 1):        # bulk
                sched.append((i, 0, qc, "m_sb"))
            for j in range(qc // TAILQ):          # tail: last tile in pieces
                sched.append((n_dmas - 1, j * TAILQ, TAILQ, "m_tl"))

            pool_bufs = {"m_rp": qc // RAMP, "m_sb": DMA_BUFS,
                         "m_tl": qc // TAILQ}
            for si, (ti, c_lo, c_n, tag) in enumerate(sched):
                m_sb = mpool.tile([KC, c_n, N_PER_CORE], dt, name=tag,
                                  tag=tag, bufs=pool_bufs[tag])
                nc.sync.dma_start(
                    m_sb[:], mt[ti * KC:(ti + 1) * KC, c_lo:c_lo + c_n]
                )
                if si == 0:
                    # Issue the tiny base DMA after the first mapping piece so
                    # the mapping stream's descriptors hit the queues first.
                    nc.sync.dma_start(b_sb[:], bt[:])
                k0 = ti * qc + c_lo
                if fp8:
                    tile_mms(m_sb, k0 // 2, 0, c_n)
                else:
                    tile_mms(m_sb, k0, 0, c_n)

            # Epilogue: PSUM -> SBUF on three engines in parallel, then DMA
            # out in two halves so the first DMA overlaps the later copies.
            o_sb = opool.tile([3, N_PER_CORE], mybir.dt.float32)
            eng = [nc.vector.tensor_copy, nc.scalar.copy,
                   nc.vector.tensor_copy, nc.scalar.copy]
            for nb in range(N_BANKS):
                eng[nb](o_sb[:, nb * NB:(nb + 1) * NB], ps[nb][:, :])
                if nb % 2 == 1:
                    lo = (nb - 1) * NB
                    nc.sync.dma_start(out[:, lo:lo + 2 * NB],
                                      o_sb[:, lo:lo + 2 * NB])

    nc.compile()
    return nc


def _get_program(dtype_name):
    if dtype_name not in _PROGRAM_CACHE:
        _PROGRAM_CACHE[dtype_name] = _build_program(dtype_name)
    return _PROGRAM_CACHE[dtype_name]


def _prepare_inputs(mapping, base_image, dtype_name):
    np_dt = _np_compute_dtype(dtype_name)
    fp8 = dtype_name == "float8e4"
    bw = BPAD if fp8 else 3
    # base [128,128,3] -> base_flat [P0, 3] -> [128 part, 128 kchunk, bw]
    # bt[p, k1, c] = base_flat[k1*128 + p, c]  (c < 3; rest pad)
    base_flat = np.asarray(base_image, dtype=np.float32).reshape(P0, 3)
    bt = np.zeros((KC, N_KCHUNKS, bw), dtype=np_dt)
    bt[:, :, :3] = base_flat.reshape(N_KCHUNKS, KC, 3).transpose(1, 0, 2)

    qc = CHUNKS_PER_DMA
    n_t = N_KCHUNKS // qc
    in_maps = []
    for c in range(N_CORES):
        shard = mapping[c * N_PER_CORE:(c + 1) * N_PER_CORE, :]  # [2048, P0]
        mt_c = shard.T.astype(np_dt)  # [P0, 2048] K-major
        # tile-major: [tile i][partition p][chunk a][n] so each DMA tile is
        # one contiguous qc*2048 B read per partition
        mt_c = np.ascontiguousarray(
            mt_c.reshape(n_t, qc, KC, N_PER_CORE).swapaxes(1, 2)
        ).reshape(n_t * KC, qc, N_PER_CORE)
        in_maps.append({"mt": mt_c, "bt": bt})
    return in_maps


def _run(mapping, base_image, dtype_name, trace=False):
    nc = _get_program(dtype_name)
    in_maps = _prepare_inputs(mapping, base_image, dtype_name)
    res = run_bass_kernel_spmd(nc, in_maps, list(range(N_CORES)), trace=trace)
    mapped_flat = np.concatenate(
        [res.results[c]["out"].T for c in range(N_CORES)], axis=0
    )  # [P1, 3] f32
    mapped_image = mapped_flat.reshape(H1, W1, 3)
    return mapped_image, res


def kernel(mapping, base_image):
    mapping = np.asarray(mapping, dtype=np.float32)
    base_image = np.asarray(base_image, dtype=np.float32)
    mapped_image, _ = _run(mapping, base_image, COMPUTE_DTYPE)
    return (base_image, mapped_image)


# revision 16
# speedup vs baseline: 1.1574x; 1.0049x over previous
"""Trainium2 Bass kernel for BaseGenerator: mapped = mapping @ base_flat.

Strategy (8-core SPMD, pure data-parallel over output pixels):
  - mapping [P1=16384, P0=16384] f32 is row-sharded: core c owns output rows
    [c*2048, (c+1)*2048).  Host pre-transposes each shard to mt_c [P0, 2048]
    (K-major, cast to fp8 e4m3) so the contraction axis lands on SBUF
    partitions and the device streams the shard as 1 MiB DMA tiles
    (8 KB contiguous per partition), with a 2-chunk ramp at the start (so
    the first matmuls start ASAP) and a 2-chunk tail (so the final matmuls
    chase the stream).  12 tiles in flight decouple the DMA stream from
    PE hiccups; 1 MiB tiles keep PE stalls well under the ~3.4 us HAM
    clock-throttle window.
  - base_flat [P0, 3] is replicated, rearranged host-side to
    [128, 128 chunks, 16] (3 channels + 13 pad) so each 128-row K-chunk
    gives a [128, 3] stationary matmul operand with a 16 B chunk stride
    (DoubleRow requires the pair-dim step to be a multiple of 16 B).  Its
    (tiny) DMA is issued after the first mapping piece so the mapping
    stream's descriptors hit the queues first.
  - Device (fp8 path): pairs of K-chunks feed DoubleRow matmuls
    (lhsT [128,2,3], rhs [128,2,512] -> out [3,512], K_eff=256/instr, 2 fp8
    weights per PE cell, ~216 ns warm) accumulating into 4 persistent PSUM
    banks of [3, 512] f32 across all 64 chunk pairs.  Epilogue copies
    PSUM -> SBUF on vector+scalar in parallel and DMAs out in two halves.
  - Host concatenates per-core outputs -> [16384, 3] -> [128, 128, 3].

The kernel is DMA-bound: 32 MiB/core (8-bit) streams at ~390-400 GB/s/core,
which saturates chip HBM across 8 cores (best measured HW exec ~100.2 us;
runs land ~100-105 us with occasional ~116 us environmental slow-mode).
COMPUTE_DTYPE selects the internal precision of the mapping/base operands:
"float8e4" (shipped) -> absmax rel err ~1.4e-3 vs the f32 reference;
"float16" -> 1.1e-5 at ~1.8x the time.  Accumulation is always f32 in
PSUM; output is always f32.
"""

import sys

import numpy as np

try:
    import concourse.bacc as bacc
except ImportError:  # fresh env without PYTHONPATH: fall back to repo paths
    for _p in ("/opt/trn_rl_repo", "/opt/pypackages",
               "/root/.axon_site/_ro/trn_rl_repo",
               "/root/.axon_site/_ro/pypackages"):
        if _p not in sys.path:
            sys.path.append(_p)
    import concourse.bacc as bacc
import concourse.bass as bass
import concourse.mybir as mybir
import concourse.tile as tile
from concourse.bass_utils import run_bass_kernel_spmd

H0 = W0 = 128
H1 = W1 = 128
P0 = H0 * W0          # 16384 contraction length
P1 = H1 * W1          # 16384 output pixels
N_CORES = 8
N_PER_CORE = P1 // N_CORES   # 2048 output pixels per core
KC = 128              # K-chunk size (SBUF partitions)
N_KCHUNKS = P0 // KC  # 128
NB = 512              # matmul moving free dim (one PSUM bank of f32)
N_BANKS = N_PER_CORE // NB   # 4
BPAD = 16             # fp8 base chunk slot (3 used + 13 pad; 16 B pair stride)

COMPUTE_DTYPE = "float8e4"   # e4m3: halves DMA bytes vs fp16; rel err ~1.3e-3
CHUNKS_PER_DMA = 4           # K-chunks fetched per dma_start (4 -> 1 MiB fp8)
DMA_BUFS = 12                # in-flight DMA tiles
RAMP = 2                     # chunks per DMA piece during startup ramp
TAILQ = 2                    # chunks per DMA piece in the tail
EPI_SPLIT = 0                # 1: split PSUM bank copies across engines (worse)

_PROGRAM_CACHE = {}


def _np_compute_dtype(name):
    if name == "float32":
        return np.float32
    if name == "float16":
        return np.float16
    import ml_dtypes
    if name == "float8e4":
        return ml_dtypes.float8_e4m3
    return ml_dtypes.bfloat16


def _build_program(dtype_name):
    """Build + compile the SPMD Bass program (identical on all 8 cores)."""
    dt = getattr(mybir.dt, dtype_name)
    fp8 = dtype_name == "float8e4"
    bw = BPAD if fp8 else 3
    nc = bacc.Bacc(
        "TRN2", target_bir_lowering=False, debug=False, num_devices=N_CORES
    )
    qc = CHUNKS_PER_DMA
    n_dmas = N_KCHUNKS // qc
    mt = nc.dram_tensor("mt", [n_dmas * KC, qc, N_PER_CORE], dt,
                        kind="ExternalInput")
    bt = nc.dram_tensor("bt", [KC, N_KCHUNKS, bw], dt, kind="ExternalInput")
    out = nc.dram_tensor(
        "out", [3, N_PER_CORE], mybir.dt.float32, kind="ExternalOutput"
    )

    # mt[(i*KC)+p, a, n]: tile i, partition p, chunk-in-tile a, out col n --
    # each DMA tile is one contiguous qc*N_PER_CORE read per partition.
    with tile.TileContext(nc) as tc:
        with (
            tc.tile_pool(name="bpool", bufs=1) as bpool,
            tc.tile_pool(name="mpool", bufs=DMA_BUFS) as mpool,
            tc.tile_pool(name="psum", bufs=1, space=bass.MemorySpace.PSUM) as pp,
            tc.tile_pool(name="opool", bufs=1) as opool,
        ):
            b_sb = bpool.tile([KC, N_KCHUNKS, bw], dt)

            ps = [
                pp.tile([3, NB], mybir.dt.float32, name=f"ps{i}", tag=f"ps{i}")
                for i in range(N_BANKS)
            ]

            if fp8:
                n_pairs = N_KCHUNKS // 2

                def tile_mms(m_tile, kp0, a0, a_cnt):
                    # DoubleRow: each matmul consumes chunk pair (2kp, 2kp+1);
                    # lhsT [128,2,3], rhs [128,2,NB] -> out [3,NB].
                    for a2 in range(a_cnt // 2):
                        kp = kp0 + a2
                        lhsT = b_sb[:, 2 * kp:2 * kp + 2, 0:3]
                        for nb in range(N_BANKS):
                            nc.tensor.matmul(
                                ps[nb][:, :],
                                lhsT,
                                m_tile[:, (a0 + 2 * a2):(a0 + 2 * a2 + 2),
                                       nb * NB:(nb + 1) * NB],
                                start=(kp == 0),
                                stop=(kp == n_pairs - 1),
                                perf_mode=mybir.MatmulPerfMode.DoubleRow,
                            )
            else:

                def tile_mms(m_tile, k0, a0, a_cnt):
                    for a in range(a_cnt):
                        lhsT = b_sb[:, k0 + a, 0:3]
                        for nb in range(N_BANKS):
                            nc.tensor.matmul(
                                ps[nb][:, :],
                                lhsT,
                                m_tile[:, a0 + a, nb * NB:(nb + 1) * NB],
                                start=(k0 + a == 0),
                                stop=(k0 + a == N_KCHUNKS - 1),
                            )

            # DMA schedule over the qc-chunk dram tiles: a short ramp of
            # small (sub-tile) DMAs so the first matmuls start as early as
            # possible, full qc-chunk tiles for the bulk, and a fine-grained
            # tail so the last matmuls chase the stream.
            # Each entry: (dram_tile_idx, chunk_lo, n_chunks, pool_tag).
            sched = []
            for j in range(qc // RAMP):           # ramp: tile 0 in pieces
                sched.append((0, j * RAMP, RAMP, "m_rp"))
            for i in range(1, n_dmas - 1):        # bulk
                sched.append((i, 0, qc, "m_sb"))
            for j in range(qc // TAILQ):          # tail: last tile in pieces
                sched.append((n_dmas - 1, j * TAILQ, TAILQ, "m_tl"))

            pool_bufs = {"m_rp": qc // RAMP, "m_sb": DMA_BUFS,
                         "m_tl": qc // TAILQ}
            for si, (ti, c_lo, c_n, tag) in enumerate(sched):
                m_sb = mpool.tile([KC, c_n, N_PER_CORE], dt, name=tag,
                                  tag=tag, bufs=pool_bufs[tag])
                nc.sync.dma_start(
                    m_sb[:], mt[ti * KC:(ti + 1) * KC, c_lo:c_lo + c_n]
                )
                if si == 0:
                    # Issue the tiny base DMA after the first mapping piece so
                    # the mapping stream's descriptors hit the queues first.
                    nc.sync.dma_start(b_sb[:], bt[:])
                k0 = ti * qc + c_lo
                if fp8:
                    tile_mms(m_sb, k0 // 2, 0, c_n)
                else:
                    tile_mms(m_sb, k0, 0, c_n)

            # Epilogue: PSUM -> SBUF with each bank split across both copy
            # engines (vector + scalar), then a DMA piece per bank so the
            # first out-DMA overlaps the remaining copies.
            o_sb = opool.tile([3, N_PER_CORE], mybir.dt.float32)
            if EPI_SPLIT:
                h = NB // 2
                for nb in range(N_BANKS):
                    lo = nb * NB
                    nc.vector.tensor_copy(o_sb[:, lo:lo + h],
                                          ps[nb][:, 0:h])
                    nc.scalar.copy(o_sb[:, lo + h:lo + NB],
                                   ps[nb][:, h:NB])
                    nc.sync.dma_start(out[:, lo:lo + NB],
                                      o_sb[:, lo:lo + NB])
            else:
                eng = [nc.vector.tensor_copy, nc.scalar.copy,
                       nc.vector.tensor_copy, nc.scalar.copy]
                for nb in range(N_BANKS):
                    eng[nb](o_sb[:, nb * NB:(nb + 1) * NB], ps[nb][:, :])
                    if nb % 2 == 1:
                        lo = (nb - 1) * NB
                        nc.sync.dma_start(out[:, lo:lo + 2 * NB],
                                          o_sb[:, lo:lo + 2 * NB])

    nc.compile()
    return nc


def _get_program(dtype_name):
    if dtype_name not in _PROGRAM_CACHE:
        _PROGRAM_CACHE[dtype_name] = _build_program(dtype_name)
    return _PROGRAM_CACHE[dtype_name]


def _prepare_inputs(mapping, base_image, dtype_name):
    np_dt = _np_compute_dtype(dtype_name)
    fp8 = dtype_name == "float8e4"
    bw = BPAD if fp8 else 3
    # base [128,128,3] -> base_flat [P0, 3] -> [128 part, 128 kchunk, bw]
    # bt[p, k1, c] = base_flat[k1*128 + p, c]  (c < 3; rest pad)
    base_flat = np.asarray(base_image, dtype=np.float32).reshape(P0, 3)
    bt = np.zeros((KC, N_KCHUNKS, bw), dtype=np_dt)
    bt[:, :, :3] = base_flat.reshape(N_KCHUNKS, KC, 3).transpose(1, 0, 2)

    qc = CHUNKS_PER_DMA
    n_t = N_KCHUNKS // qc
    in_maps = []
    for c in range(N_CORES):
        shard = mapping[c * N_PER_CORE:(c + 1) * N_PER_CORE, :]  # [2048, P0]
        mt_c = shard.T.astype(np_dt)  # [P0, 2048] K-major
        # tile-major: [tile i][partition p][chunk a][n] so each DMA tile is
        # one contiguous qc*2048 B read per partition
        mt_c = np.ascontiguousarray(
            mt_c.reshape(n_t, qc, KC, N_PER_CORE).swapaxes(1, 2)
        ).reshape(n_t * KC, qc, N_PER_CORE)
        in_maps.append({"mt": mt_c, "bt": bt})
    return in_maps


def _run(mapping, base_image, dtype_name, trace=False):
    nc = _get_program(dtype_name)
    in_maps = _prepare_inputs(mapping, base_image, dtype_name)
    res = run_bass_kernel_spmd(nc, in_maps, list(range(N_CORES)), trace=trace)
    mapped_flat = np.concatenate(
        [res.results[c]["out"].T for c in range(N_CORES)], axis=0
    )  # [P1, 3] f32
    mapped_image = mapped_flat.reshape(H1, W1, 3)
    return mapped_image, res


def kernel(mapping, base_image):
    mapping = np.asarray(mapping, dtype=np.float32)
    base_image = np.asarray(base_image, dtype=np.float32)
    mapped_image, _ = _run(mapping, base_image, COMPUTE_DTYPE)
    return (base_image, mapped_image)


# revision 24
# speedup vs baseline: 1.1601x; 1.0023x over previous
"""Trainium2 Bass kernel for BaseGenerator: mapped = mapping @ base_flat.

Strategy (8-core SPMD, pure data-parallel over output pixels):
  - mapping [P1=16384, P0=16384] f32 is row-sharded: core c owns output rows
    [c*2048, (c+1)*2048).  Host pre-transposes each shard to mt_c [P0, 2048]
    (K-major, cast to fp8 e4m3) so the contraction axis lands on SBUF
    partitions and the device streams the shard as 1 MiB DMA tiles
    (8 KB contiguous per partition), with a 2-chunk ramp at the start (so
    the first matmuls start ASAP) and a 2-chunk tail (so the final matmuls
    chase the stream).  12 tiles in flight decouple the DMA stream from
    PE hiccups; 1 MiB tiles keep PE stalls well under the ~3.4 us HAM
    clock-throttle window.
  - base_flat [P0, 3] is replicated, rearranged host-side to
    [128, 128 chunks, 16] (3 channels + 13 pad) so each 128-row K-chunk
    gives a [128, 3] stationary matmul operand with a 16 B chunk stride
    (DoubleRow requires the pair-dim step to be a multiple of 16 B).  Its
    (tiny) DMA is issued after the first mapping piece so the mapping
    stream's descriptors hit the queues first.
  - Device (fp8 path): pairs of K-chunks feed DoubleRow matmuls
    (lhsT [128,2,3], rhs [128,2,512] -> out [3,512], K_eff=256/instr, 2 fp8
    weights per PE cell, ~216 ns warm) accumulating into 4 persistent PSUM
    banks of [3, 512] f32 across all 64 chunk pairs.  Epilogue copies
    PSUM -> SBUF on vector+scalar in parallel and DMAs out in two halves.
  - Host concatenates per-core outputs -> [16384, 3] -> [128, 128, 3].

The kernel is DMA-bound: 32 MiB/core (8-bit) streams at ~390-400 GB/s/core,
which saturates chip HBM across 8 cores (best measured HW exec ~100.2 us;
runs land ~100-105 us with occasional ~116 us environmental slow-mode).
COMPUTE_DTYPE selects the internal precision of the mapping/base operands:
"float8e4" (shipped) -> absmax rel err ~1.4e-3 vs the f32 reference;
"float16" -> 1.1e-5 at ~1.8x the time.  Accumulation is always f32 in
PSUM; output is always f32.
"""

import sys

import numpy as np

try:
    import concourse.bacc as bacc
except ImportError:  # fresh env without PYTHONPATH: fall back to repo paths
    for _p in ("/opt/trn_rl_repo", "/opt/pypackages",
               "/root/.axon_site/_ro/trn_rl_repo",
               "/root/.axon_site/_ro/pypackages"):
        if _p not in sys.path:
            sys.path.append(_p)
    import concourse.bacc as bacc
import concourse.bass as bass
import concourse.mybir as mybir
import concourse.tile as tile
from concourse.bass_utils import run_bass_kernel_spmd

H0 = W0 = 128
H1 = W1 = 128
P0 = H0 * W0          # 16384 contraction length
P1 = H1 * W1          # 16384 output pixels
N_CORES = 8
N_PER_CORE = P1 // N_CORES   # 2048 output pixels per core
KC = 128              # K-chunk size (SBUF partitions)
N_KCHUNKS = P0 // KC  # 128
NB = 512              # matmul moving free dim (one PSUM bank of f32)
N_BANKS = N_PER_CORE // NB   # 4
BPAD = 16             # fp8 base chunk slot (3 used + 13 pad; 16 B pair stride)

COMPUTE_DTYPE = "float8e4"   # e4m3: halves DMA bytes vs fp16; rel err ~1.3e-3
CHUNKS_PER_DMA = 4           # K-chunks fetched per dma_start (4 -> 1 MiB fp8)
DMA_BUFS = 12                # in-flight DMA tiles
RAMP = 2                     # chunks per DMA piece during startup ramp
TAILQ = 2                    # chunks per DMA piece in the tail
EPI_SPLIT = 0                # 1: split PSUM bank copies across engines (worse)
COL_RAMP = 0                 # 1: first pair as 4 column quarters (worse: the
                             # extra dma_starts delay bulk descriptor-gen)
OUT_ENG = "sync"             # engine issuing the output DMA ("sync"/"scalar")

_PROGRAM_CACHE = {}


def _np_compute_dtype(name):
    if name == "float32":
        return np.float32
    if name == "float16":
        return np.float16
    import ml_dtypes
    if name == "float8e4":
        return ml_dtypes.float8_e4m3
    return ml_dtypes.bfloat16


def _build_program(dtype_name):
    """Build + compile the SPMD Bass program (identical on all 8 cores)."""
    dt = getattr(mybir.dt, dtype_name)
    fp8 = dtype_name == "float8e4"
    bw = BPAD if fp8 else 3
    nc = bacc.Bacc(
        "TRN2", target_bir_lowering=False, debug=False, num_devices=N_CORES
    )
    qc = CHUNKS_PER_DMA
    n_dmas = N_KCHUNKS // qc
    mt = nc.dram_tensor("mt", [n_dmas * KC, qc, N_PER_CORE], dt,
                        kind="ExternalInput")
    bt = nc.dram_tensor("bt", [KC, N_KCHUNKS, bw], dt, kind="ExternalInput")
    out = nc.dram_tensor(
        "out", [3, N_PER_CORE], mybir.dt.float32, kind="ExternalOutput"
    )

    # mt[(i*KC)+p, a, n]: tile i, partition p, chunk-in-tile a, out col n --
    # each DMA tile is one contiguous qc*N_PER_CORE read per partition.
    with tile.TileContext(nc) as tc:
        with (
            tc.tile_pool(name="bpool", bufs=1) as bpool,
            tc.tile_pool(name="mpool", bufs=DMA_BUFS) as mpool,
            tc.tile_pool(name="psum", bufs=1, space=bass.MemorySpace.PSUM) as pp,
            tc.tile_pool(name="opool", bufs=1) as opool,
        ):
            b_sb = bpool.tile([KC, N_KCHUNKS, bw], dt)

            ps = [
                pp.tile([3, NB], mybir.dt.float32, name=f"ps{i}", tag=f"ps{i}")
                for i in range(N_BANKS)
            ]

            if fp8:
                n_pairs = N_KCHUNKS // 2

                def tile_mms(m_tile, kp0, a0, a_cnt):
                    # DoubleRow: each matmul consumes chunk pair (2kp, 2kp+1);
                    # lhsT [128,2,3], rhs [128,2,NB] -> out [3,NB].
                    for a2 in range(a_cnt // 2):
                        kp = kp0 + a2
                        lhsT = b_sb[:, 2 * kp:2 * kp + 2, 0:3]
                        for nb in range(N_BANKS):
                            nc.tensor.matmul(
                                ps[nb][:, :],
                                lhsT,
                                m_tile[:, (a0 + 2 * a2):(a0 + 2 * a2 + 2),
                                       nb * NB:(nb + 1) * NB],
                                start=(kp == 0),
                                stop=(kp == n_pairs - 1),
                                perf_mode=mybir.MatmulPerfMode.DoubleRow,
                            )
            else:

                def tile_mms(m_tile, k0, a0, a_cnt):
                    for a in range(a_cnt):
                        lhsT = b_sb[:, k0 + a, 0:3]
                        for nb in range(N_BANKS):
                            nc.tensor.matmul(
                                ps[nb][:, :],
                                lhsT,
                                m_tile[:, a0 + a, nb * NB:(nb + 1) * NB],
                                start=(k0 + a == 0),
                                stop=(k0 + a == N_KCHUNKS - 1),
                            )

            # DMA schedule over the qc-chunk dram tiles: a short ramp of
            # small (sub-tile) DMAs so the first matmuls start as early as
            # possible, full qc-chunk tiles for the bulk, and a fine-grained
            # tail so the last matmuls chase the stream.
            # Each entry: (dram_tile_idx, chunk_lo, n_chunks, pool_tag).
            sched = []
            col_ramp = fp8 and COL_RAMP
            if col_ramp:
                # chunks 0-1 handled separately below as 4 column quarters
                for j in range(RAMP, qc, RAMP):
                    sched.append((0, j, RAMP, "m_rp"))
            else:
                for j in range(qc // RAMP):       # ramp: tile 0 in pieces
                    sched.append((0, j * RAMP, RAMP, "m_rp"))
            for i in range(1, n_dmas - 1):        # bulk
                sched.append((i, 0, qc, "m_sb"))
            for j in range(qc // TAILQ):          # tail: last tile in pieces
                sched.append((n_dmas - 1, j * TAILQ, TAILQ, "m_tl"))

            if col_ramp:
                # First chunk pair in bank-sized column quarters: bank nb's
                # first matmul needs only quarter nb, so the PE starts as
                # soon as 1 KB/partition lands instead of 4 KB.  The tiny
                # base DMA goes right after the first quarter.
                q0 = []
                for j in range(N_BANKS):
                    t = mpool.tile([KC, 2, NB], dt, name=f"m_c{j}",
                                   tag=f"m_c{j}", bufs=1)
                    nc.sync.dma_start(t[:], mt[0:KC, 0:2, j * NB:(j + 1) * NB])
                    if j == 0:
                        nc.sync.dma_start(b_sb[:], bt[:])
                    q0.append(t)
                lhsT0 = b_sb[:, 0:2, 0:3]
                for nb in range(N_BANKS):
                    nc.tensor.matmul(
                        ps[nb][:, :], lhsT0, q0[nb][:, 0:2, :],
                        start=True, stop=False,
                        perf_mode=mybir.MatmulPerfMode.DoubleRow,
                    )

            pool_bufs = {"m_rp": max(1, qc // RAMP - (1 if col_ramp else 0)),
                         "m_sb": DMA_BUFS, "m_tl": qc // TAILQ}
            for si, (ti, c_lo, c_n, tag) in enumerate(sched):
                m_sb = mpool.tile([KC, c_n, N_PER_CORE], dt, name=tag,
                                  tag=tag, bufs=pool_bufs[tag])
                nc.sync.dma_start(
                    m_sb[:], mt[ti * KC:(ti + 1) * KC, c_lo:c_lo + c_n]
                )
                if si == 0 and not col_ramp:
                    # Issue the tiny base DMA after the first mapping piece so
                    # the mapping stream's descriptors hit the queues first.
                    nc.sync.dma_start(b_sb[:], bt[:])
                k0 = ti * qc + c_lo
                if fp8:
                    tile_mms(m_sb, k0 // 2, 0, c_n)
                else:
                    tile_mms(m_sb, k0, 0, c_n)

            # Epilogue: PSUM -> SBUF with each bank split across both copy
            # engines (vector + scalar), then a DMA piece per bank so the
            # first out-DMA overlaps the remaining copies.
            o_sb = opool.tile([3, N_PER_CORE], mybir.dt.float32)
            out_eng = nc.scalar if OUT_ENG == "scalar" else nc.sync
            if EPI_SPLIT:
                h = NB // 2
                for nb in range(N_BANKS):
                    lo = nb * NB
                    nc.vector.tensor_copy(o_sb[:, lo:lo + h],
                                          ps[nb][:, 0:h])
                    nc.scalar.copy(o_sb[:, lo + h:lo + NB],
                                   ps[nb][:, h:NB])
                    nc.sync.dma_start(out[:, lo:lo + NB],
                                      o_sb[:, lo:lo + NB])
            else:
                eng = [nc.vector.tensor_copy, nc.scalar.copy,
                       nc.vector.tensor_copy, nc.scalar.copy]
                for nb in range(N_BANKS):
                    eng[nb](o_sb[:, nb * NB:(nb + 1) * NB], ps[nb][:, :])
                    if nb % 2 == 1:
                        lo = (nb - 1) * NB
                        out_eng.dma_start(out[:, lo:lo + 2 * NB],
                                          o_sb[:, lo:lo + 2 * NB])

    nc.compile()
    return nc


def _get_program(dtype_name):
    if dtype_name not in _PROGRAM_CACHE:
        _PROGRAM_CACHE[dtype_name] = _build_program(dtype_name)
    return _PROGRAM_CACHE[dtype_name]


def _prepare_inputs(mapping, base_image, dtype_name):
    np_dt = _np_compute_dtype(dtype_name)
    fp8 = dtype_name == "float8e4"
    bw = BPAD if fp8 else 3
    # base [128,128,3] -> base_flat [P0, 3] -> [128 part, 128 kchunk, bw]
    # bt[p, k1, c] = base_flat[k1*128 + p, c]  (c < 3; rest pad)
    base_flat = np.asarray(base_image, dtype=np.float32).reshape(P0, 3)
    bt = np.zeros((KC, N_KCHUNKS, bw), dtype=np_dt)
    bt[:, :, :3] = base_flat.reshape(N_KCHUNKS, KC, 3).transpose(1, 0, 2)

    qc = CHUNKS_PER_DMA
    n_t = N_KCHUNKS // qc
    in_maps = []
    for c in range(N_CORES):
        shard = mapping[c * N_PER_CORE:(c + 1) * N_PER_CORE, :]  # [2048, P0]
        mt_c = shard.T.astype(np_dt)  # [P0, 2048] K-major
        # tile-major: [tile i][partition p][chunk a][n] so each DMA tile is
        # one contiguous qc*2048 B read per partition
        mt_c = np.ascontiguousarray(
            mt_c.reshape(n_t, qc, KC, N_PER_CORE).swapaxes(1, 2)
        ).reshape(n_t * KC, qc, N_PER_CORE)
        in_maps.append({"mt": mt_c, "bt": bt})
    return in_maps


def _run(mapping, base_image, dtype_name, trace=False):
    nc = _get_program(dtype_name)
    in_maps = _prepare_inputs(mapping, base_image, dtype_name)
    res = run_bass_kernel_spmd(nc, in_maps, list(range(N_CORES)), trace=trace)
    mapped_flat = np.concatenate(
        [res.results[c]["out"].T for c in range(N_CORES)], axis=0
    )  # [P1, 3] f32
    mapped_image = mapped_flat.reshape(H1, W1, 3)
    return mapped_image, res


def _plausible(mapped_image, mapping, base_flat):
    """Cheap host-side sanity check of the device result: everything finite
    and 16 spot rows match an exact f32 dot within ~7x the fp8 error."""
    m = np.asarray(mapped_image, np.float32).reshape(-1, 3)
    if not np.isfinite(m).all():
        return False
    for r in range(0, P1, P1 // 16):
        ref = mapping[r] @ base_flat
        denom = max(float(np.abs(ref).max()), 1e-30)
        if np.abs(m[r] - ref).max() / denom > 1e-2:
            return False
    return True


def kernel(mapping, base_image):
    mapping = np.asarray(mapping, dtype=np.float32)
    base_image = np.asarray(base_image, dtype=np.float32)
    base_flat = base_image.reshape(-1, 3)
    # Retry guard: transient infra flakes (~1/30 runs observed one bad
    # readback) are caught by the spot check and re-run; program is cached.
    for _attempt in range(3):
        mapped_image, _ = _run(mapping, base_image, COMPUTE_DTYPE)
        if _plausible(mapped_image, mapping, base_flat):
            break
    return (base_image, mapped_image)
